# revision 11
# baseline (speedup 1.0000x reference)
"""Trainium2 Bass kernel for a 12-layer EVA-style ViT encoder (B=16, N=256, D=768).

Sharding: pure data-parallel over batch across 8 NeuronCores (2 images/core).
Per core: feature-major activations [feature, token] (T=512 token columns),
all matmuls float32r (11-bit mantissa at bf16 speed), fp32 residual stream,
LayerNorm affines & biases folded on host (identity/zero for this model).
"""
import sys, types

sys.path.insert(0, '/opt/trn_rl_repo')

import numpy as np

B, NTOK, DIM, HEADS, HD, DEPTH, HIDDEN = 16, 256, 768, 12, 64, 12, 2048
EPS = 1e-5
NCORES = 8
BPC = B // NCORES          # batch items per core
T = BPC * NTOK             # 512 token columns per core
KD = DIM // 128            # 6
KH = HIDDEN // 128         # 16
SCALE = HD ** -0.5

_CACHE = {}


def _install_ntff_shim():
    if "antenv.axon_hooks" in sys.modules:
        return
    m = types.ModuleType("antenv.axon_hooks")
    m._hook = None
    m.set_axon_ntff_profile_hook = lambda h: setattr(m, "_hook", h)
    m.get_axon_ntff_profile_hook = lambda: m._hook
    sys.modules["antenv.axon_hooks"] = m
    try:
        from trn_agent_boot.trn_boot import _ntff_profile_via_ctypes
        m.set_axon_ntff_profile_hook(_ntff_profile_via_ctypes('/opt/axon/libaxon_pjrt.so'))
    except Exception:
        pass


def _build(layers=DEPTH, dbg=False):
    import concourse.bass as bass
    import concourse.mybir as mybir
    import concourse.tile as tile
    from concourse import bacc
    from contextlib import ExitStack

    f32 = mybir.dt.float32
    f32r = mybir.dt.float32r
    i32 = mybir.dt.int32
    AF = mybir.ActivationFunctionType
    OP = mybir.AluOpType

    nc = bacc.Bacc("TRN2", target_bir_lowering=False, debug=False)

    x_fm = nc.dram_tensor("x_fm", [DIM, T], f32r, kind="ExternalInput")
    WQ = nc.dram_tensor("WQ", [layers, DIM, DIM], f32r, kind="ExternalInput")
    WK = nc.dram_tensor("WK", [layers, DIM, DIM], f32r, kind="ExternalInput")
    WV = nc.dram_tensor("WV", [layers, DIM, DIM], f32r, kind="ExternalInput")
    WO = nc.dram_tensor("WO", [layers, DIM, DIM], f32r, kind="ExternalInput")
    W1G = nc.dram_tensor("W1G", [layers, DIM, HIDDEN], f32r, kind="ExternalInput")
    W1X = nc.dram_tensor("W1X", [layers, DIM, HIDDEN], f32r, kind="ExternalInput")
    W2 = nc.dram_tensor("W2", [layers, HIDDEN, DIM], f32r, kind="ExternalInput")
    SIN = nc.dram_tensor("SIN", [128, T], f32, kind="ExternalInput")
    COS = nc.dram_tensor("COS", [128, T], f32, kind="ExternalInput")
    PERM = nc.dram_tensor("PERM", [128, 128], f32r, kind="ExternalInput")
    ONES = nc.dram_tensor("ONES", [1, 128], f32r, kind="ExternalInput")
    out_fm = nc.dram_tensor("out_fm", [DIM, T], f32, kind="ExternalOutput")
    if dbg:
        DY = nc.dram_tensor("DY", [DIM, T], f32, kind="ExternalOutput")
        DQ = nc.dram_tensor("DQ", [DIM, T], f32, kind="ExternalOutput")
        DK = nc.dram_tensor("DK", [DIM, T], f32, kind="ExternalOutput")
        DV = nc.dram_tensor("DV", [512, DIM], f32, kind="ExternalOutput")
        DE = nc.dram_tensor("DE", [128, 512], f32, kind="ExternalOutput")
        DOT = nc.dram_tensor("DOT", [DIM, T], f32, kind="ExternalOutput")
        DH = nc.dram_tensor("DH", [DIM, T], f32, kind="ExternalOutput")
        DS = nc.dram_tensor("DS", [HIDDEN, T], f32, kind="ExternalOutput")

    with tile.TileContext(nc) as tc:
        with ExitStack() as ctx:
            ctx.enter_context(nc.allow_low_precision(
                reason="float32r tiles hold full fp32 bits; rounding only at PE read"))
            const = ctx.enter_context(tc.tile_pool(name="const", bufs=1))
            hp = ctx.enter_context(tc.tile_pool(name="hp", bufs=1))
            yp = ctx.enter_context(tc.tile_pool(name="yp", bufs=1))
            sqp = ctx.enter_context(tc.tile_pool(name="sqp", bufs=1))
            rowp = ctx.enter_context(tc.tile_pool(name="rowp", bufs=1))
            qrawp = ctx.enter_context(tc.tile_pool(name="qrawp", bufs=2))
            ropep = ctx.enter_context(tc.tile_pool(name="ropep", bufs=1))
            rtmp = ctx.enter_context(tc.tile_pool(name="rtmp", bufs=2))
            vp = ctx.enter_context(tc.tile_pool(name="vp", bufs=1))
            ep = ctx.enter_context(tc.tile_pool(name="ep", bufs=2))
            recp = ctx.enter_context(tc.tile_pool(name="recp", bufs=2))
            op_ = ctx.enter_context(tc.tile_pool(name="op", bufs=1))
            sp = ctx.enter_context(tc.tile_pool(name="sp", bufs=2))
            snp = ctx.enter_context(tc.tile_pool(name="snp", bufs=1))
            wp = ctx.enter_context(tc.tile_pool(name="wp", bufs=2))
            wvp = ctx.enter_context(tc.tile_pool(name="wvp", bufs=1))
            wmp = ctx.enter_context(tc.tile_pool(name="wmp", bufs=3))
            PS = ctx.enter_context(tc.tile_pool(name="PS", bufs=1, space="PSUM"))

            def pst(tag, shape=None, dt_=f32, name=None):
                return PS.tile(shape or [128, T], dt_, name=name or f"ps_{tag}_{nc.next_id()}",
                               tag=tag)

            # constants
            ones1 = const.tile([128, 1], f32r)
            nc.gpsimd.dma_start(out=ones1, in_=bass.AP(ONES.ap().tensor, ONES.ap().offset,
                                                       [[0, 128], [1, 1]]))
            onesk = const.tile([1, 128], f32r)
            nc.sync.dma_start(out=onesk, in_=ONES[:, :])
            magic = const.tile([1, T], i32)
            nc.vector.memset(magic, 0x5f3759df)
            warm8 = const.tile([128, 8], mybir.dt.bfloat16)
            nc.vector.memset(warm8, 1.0)
            sin_sb = const.tile([128, T], f32)
            nc.sync.dma_start(out=sin_sb, in_=SIN[:, :])
            cos_sb = const.tile([128, T], f32)
            nc.sync.dma_start(out=cos_sb, in_=COS[:, :])
            perm_sb = const.tile([128, 128], f32r)
            nc.sync.dma_start(out=perm_sb, in_=PERM[:, :])

            # PE warmup (absorbs entry-barrier waits)
            wps = pst("p7", [8, 8], name="warmps")
            nc.tensor.matmul(wps, warm8[:, :], warm8[:, 0:8], start=True, stop=True)

            h = []
            for k in range(KD):
                t_ = hp.tile([128, T], f32r, name=f"h_{k}", tag=f"h{k}")
                nc.sync.dma_start(out=t_, in_=x_fm[128 * k:128 * (k + 1), :])
                h.append(t_)

            def ln_chain(srow, qrow, D, sfx):
                """DVE scalar chain on [1,T] rows -> (arow=rstd f32r, bneg=-mean f32r)."""
                inv = 1.0 / D
                bneg = rowp.tile([1, T], f32r, name=f"bneg{sfx}", tag="bneg")
                nc.vector.tensor_scalar(bneg[:, :], srow[:, :], -inv, None, op0=OP.mult)
                msq = rowp.tile([1, T], f32, name=f"msq{sfx}", tag="msq")
                nc.vector.tensor_mul(msq[:, :], bneg[:, :], bneg[:, :])
                ve = rowp.tile([1, T], f32, name=f"ve{sfx}", tag="ve")
                nc.vector.scalar_tensor_tensor(ve[:, :], qrow[:, :], inv, msq[:, :],
                                               op0=OP.mult, op1=OP.subtract)
                nc.vector.tensor_scalar(ve[:, :], ve[:, :], EPS, None, op0=OP.add)
                yv = rowp.tile([1, T], i32, name=f"yv{sfx}", tag="yv")
                nc.vector.tensor_scalar(yv[:, :], ve[:, :].bitcast(i32), 1, None,
                                        op0=OP.logical_shift_right)
                nc.vector.scalar_tensor_tensor(yv[:, :], magic[:, :], 1, yv[:, :],
                                               op0=OP.mult, op1=OP.subtract)
                yf = yv[:, :].bitcast(f32)
                tt = rowp.tile([1, T], f32, name=f"tt{sfx}", tag="tt")
                uu = rowp.tile([1, T], f32, name=f"uu{sfx}", tag="uu")
                arow = rowp.tile([1, T], f32r, name=f"arow{sfx}", tag="arow")
                for it in range(2):
                    nc.vector.tensor_mul(tt[:, :], yf, yf)
                    nc.vector.tensor_mul(uu[:, :], tt[:, :], ve[:, :])
                    nc.vector.tensor_scalar(uu[:, :], uu[:, :], -0.5, 1.5,
                                            op0=OP.mult, op1=OP.add)
                    if it == 0:
                        nc.vector.tensor_mul(yv[:, :].bitcast(f32), yf, uu[:, :])
                    else:
                        nc.vector.tensor_mul(arow[:, :], yf, uu[:, :])
                return arow, bneg

            for l in range(layers):
                # ---------------- LN1 ----------------
                sqs = []
                for k in range(KD):
                    s_ = sqp.tile([128, T], f32r, name=f"sq1_{l}_{k}", tag=f"sq{k % 2}")
                    nc.scalar.activation(s_[:, :], h[k][:, :], AF.Square)
                    sqs.append(s_)
                srow = pst("p0", [1, T], name=f"srow1_{l}")
                qrow = pst("p1", [1, T], name=f"qrow1_{l}")
                for k in range(KD):
                    nc.tensor.matmul(srow, ones1[:, :], h[k][:, :],
                                     start=(k == 0), stop=(k == KD - 1))
                for k in range(KD):
                    nc.tensor.matmul(qrow, ones1[:, :], sqs[k][:, :],
                                     start=(k == 0), stop=(k == KD - 1))
                arow, bneg = ln_chain(srow, qrow, DIM, f"a{l}")
                A1 = pst("p2", name=f"A1_{l}")
                nc.tensor.matmul(A1, onesk[:, :], arow[:, :], start=True, stop=True)
                B1 = pst("p3", name=f"B1_{l}")
                nc.tensor.matmul(B1, onesk[:, :], bneg[:, :], start=True, stop=True)
                y1 = []
                for k in range(KD):
                    tmp = rtmp.tile([128, T], f32, name=f"l1t{l}_{k}", tag="lnt")
                    nc.vector.tensor_add(tmp[:, :], h[k][:, :], B1[:, :])
                    y_ = yp.tile([128, T], f32r, name=f"y1_{l}_{k}", tag=f"y{k}")
                    nc.vector.tensor_mul(y_[:, :], tmp[:, :], A1[:, :])
                    y1.append(y_)

                # ---------------- Q,K projections ----------------
                PROJ_TAGS_QK = ["p4", "p5", "p6", "p7", "p0", "p1"]
                qk_sb = {}
                for wname, Wd in (("q", WQ), ("k", WK)):
                    pss = [pst(PROJ_TAGS_QK[m], name=f"ps{wname}{l}_{m}") for m in range(KD)]
                    for k in range(KD):
                        wk_t = wp.tile([128, DIM], f32r, name=f"w{wname}{l}_{k}", tag=f"w{k % 2}")
                        nc.sync.dma_start(out=wk_t, in_=Wd[l, 128 * k:128 * (k + 1), :])
                        for m in range(KD):
                            nc.tensor.matmul(pss[m], wk_t[:, 128 * m:128 * (m + 1)],
                                             y1[k][:, :], start=(k == 0), stop=(k == KD - 1))
                    outs = []
                    for m in range(KD):
                        q_ = qrawp.tile([128, T], f32r, name=f"{wname}sb{l}_{m}", tag=f"qr{m % 2}")
                        nc.scalar.copy(q_[:, :], pss[m][:, :])
                        outs.append(q_)
                    qk_sb[wname] = outs

                # ---------------- V projection (token-major + ones cols) ----------------
                wv_t = []
                for k in range(KD):
                    wv_ = wvp.tile([128, DIM], f32r, name=f"wv{l}_{k}", tag=f"wv{k}")
                    nc.sync.dma_start(out=wv_, in_=WV[l, 128 * k:128 * (k + 1), :])
                    wv_t.append(wv_)
                vtm = []
                for mt in range(4):
                    psvA = pst("p2", [128, 384], name=f"psvA{l}_{mt}")
                    psvB = pst("p3", [128, 384], name=f"psvB{l}_{mt}")
                    for k in range(KD):
                        nc.tensor.matmul(psvA, y1[k][:, 128 * mt:128 * (mt + 1)],
                                         wv_t[k][:, 0:384], start=(k == 0), stop=(k == KD - 1))
                    for k in range(KD):
                        nc.tensor.matmul(psvB, y1[k][:, 128 * mt:128 * (mt + 1)],
                                         wv_t[k][:, 384:768], start=(k == 0), stop=(k == KD - 1))
                    v_ = vp.tile([128, 768], f32r, name=f"vtm{l}_{mt}", tag=f"v{mt}")
                    nc.scalar.copy(v_[:, 0:384], psvA[:, :])
                    nc.scalar.copy(v_[:, 384:768], psvB[:, :])
                    vtm.append(v_)

                # ---------------- RoPE ----------------
                for wname in ("q", "k"):
                    raw = qk_sb[wname]
                    roped = []
                    for m in range(KD):
                        rot = pst("p2" if m % 2 == 0 else "p3", name=f"rot{wname}{l}_{m}")
                        nc.tensor.matmul(rot, perm_sb[:, :], raw[m][:, :],
                                         start=True, stop=True)
                        t1 = rtmp.tile([128, T], f32, name=f"t1{wname}{l}_{m}", tag="t1")
                        nc.vector.tensor_mul(t1[:, :], rot[:, :], sin_sb[:, :])
                        t2 = rtmp.tile([128, T], f32, name=f"t2{wname}{l}_{m}", tag="t2")
                        nc.gpsimd.tensor_mul(t2[:, :], raw[m][:, :], cos_sb[:, :])
                        rp = ropep.tile([128, T], f32r, name=f"{wname}p{l}_{m}",
                                        tag=f"{wname}p{m}")
                        nc.vector.tensor_add(rp[:, :], t1[:, :], t2[:, :])
                        roped.append(rp)
                    qk_sb[wname] = roped
                qs, ks = qk_sb["q"], qk_sb["k"]
                if dbg and l == 0:
                    for k in range(KD):
                        nc.sync.dma_start(out=DY[128 * k:128 * (k + 1), :], in_=y1[k][:, :].bitcast(f32))
                        nc.sync.dma_start(out=DQ[128 * k:128 * (k + 1), :], in_=qs[k][:, :].bitcast(f32))
                        nc.sync.dma_start(out=DK[128 * k:128 * (k + 1), :], in_=ks[k][:, :].bitcast(f32))
                    for mt in range(4):
                        nc.sync.dma_start(out=DV[128 * mt:128 * (mt + 1), :], in_=vtm[mt][:, 0:768].bitcast(f32))

                # ---------------- attention ----------------
                o_sb = [op_.tile([128, T], f32r, name=f"osb{l}_{m}", tag=f"o{m}")
                        for m in range(KD)]
                for i in range(BPC):
                    for hh in range(HEADS):
                        p, off = hh // 2, 64 * (hh % 2)
                        psS = pst("p6", [128, 2 * NTOK], name=f"psS{l}_{i}_{hh}")
                        for kt in range(2):
                            nc.tensor.matmul(
                                psS[:, NTOK * kt:NTOK * (kt + 1)],
                                ks[p][off:off + 64,
                                      256 * i + 128 * kt:256 * i + 128 * (kt + 1)],
                                qs[p][off:off + 64, 256 * i:256 * (i + 1)],
                                start=True, stop=True)
                        eT = ep.tile([128, 2 * NTOK], f32r, name=f"eT{l}_{i}_{hh}", tag="eT")
                        nc.scalar.activation(eT[:, :], psS[:, :], AF.Exp)
                        if dbg and l == 0 and i == 0 and hh == 0:
                            nc.sync.dma_start(out=DE[:, :], in_=eT[:, :].bitcast(f32))
                        # denominator row = colsum of exp (over kt partitions)
                        psD = pst("p0", [1, NTOK], name=f"psD{l}_{i}_{hh}")
                        for kt in range(2):
                            nc.tensor.matmul(psD, ones1[:, :],
                                             eT[:, NTOK * kt:NTOK * (kt + 1)],
                                             start=(kt == 0), stop=(kt == 1))
                        recr = recp.tile([1, NTOK], f32r, name=f"recr{l}_{i}_{hh}", tag="recr")
                        nc.vector.reciprocal(recr[:, :], psD[:, :])
                        psR = pst("p1", [64, NTOK], name=f"psR{l}_{i}_{hh}")
                        nc.tensor.matmul(psR, onesk[:, 0:64], recr[:, :],
                                         start=True, stop=True)
                        rec = recp.tile([64, NTOK], f32, name=f"rec{l}_{i}_{hh}", tag="rec")
                        nc.scalar.copy(rec[:, :], psR[:, :])
                        psO = pst("p7", [64, NTOK], name=f"psO{l}_{i}_{hh}")
                        for kt in range(2):
                            vt = vtm[2 * i + kt]
                            nc.tensor.matmul(psO, vt[:, 64 * hh:64 * hh + 64],
                                             eT[:, NTOK * kt:NTOK * (kt + 1)],
                                             start=(kt == 0), stop=(kt == 1))
                        nc.vector.tensor_mul(
                            o_sb[p][off:off + 64, 256 * i:256 * (i + 1)],
                            psO[:, :], rec[:, :])

                # ---------------- O projection + residual ----------------
                PROJ_TAGS_O = ["p0", "p1", "p2", "p3", "p4", "p5"]
                pss = [pst(PROJ_TAGS_O[m], name=f"psh{l}_{m}") for m in range(KD)]
                for k in range(KD):
                    wo_t = wp.tile([128, DIM], f32r, name=f"wo{l}_{k}", tag=f"w{k % 2}")
                    nc.sync.dma_start(out=wo_t, in_=WO[l, 128 * k:128 * (k + 1), :])
                    for m in range(KD):
                        nc.tensor.matmul(pss[m], wo_t[:, 128 * m:128 * (m + 1)],
                                         o_sb[k][:, :], start=(k == 0), stop=(k == KD - 1))
                for m in range(KD):
                    nc.vector.tensor_add(h[m][:, :], h[m][:, :], pss[m][:, :])

                if dbg and l == 0:
                    for k in range(KD):
                        nc.sync.dma_start(out=DOT[128 * k:128 * (k + 1), :], in_=o_sb[k][:, :].bitcast(f32))
                        nc.sync.dma_start(out=DH[128 * k:128 * (k + 1), :], in_=h[k][:, :].bitcast(f32))
                # ---------------- LN2 ----------------
                sqs = []
                for k in range(KD):
                    s_ = sqp.tile([128, T], f32r, name=f"sq2_{l}_{k}", tag=f"sq{k % 2}")
                    nc.scalar.activation(s_[:, :], h[k][:, :], AF.Square)
                    sqs.append(s_)
                srow = pst("p6", [1, T], name=f"srow2_{l}")
                qrow = pst("p7", [1, T], name=f"qrow2_{l}")
                for k in range(KD):
                    nc.tensor.matmul(srow, ones1[:, :], h[k][:, :],
                                     start=(k == 0), stop=(k == KD - 1))
                for k in range(KD):
                    nc.tensor.matmul(qrow, ones1[:, :], sqs[k][:, :],
                                     start=(k == 0), stop=(k == KD - 1))
                arow, bneg = ln_chain(srow, qrow, DIM, f"b{l}")
                A2 = pst("p2", name=f"A2_{l}")
                nc.tensor.matmul(A2, onesk[:, :], arow[:, :], start=True, stop=True)
                B2 = pst("p3", name=f"B2_{l}")
                nc.tensor.matmul(B2, onesk[:, :], bneg[:, :], start=True, stop=True)
                y2 = []
                for k in range(KD):
                    tmp = rtmp.tile([128, T], f32, name=f"l2t{l}_{k}", tag="lnt")
                    nc.vector.tensor_add(tmp[:, :], h[k][:, :], B2[:, :])
                    y_ = yp.tile([128, T], f32r, name=f"y2_{l}_{k}", tag=f"y{k}")
                    nc.vector.tensor_mul(y_[:, :], tmp[:, :], A2[:, :])
                    y2.append(y_)

                # ---------------- MLP G/U in chunks of 2 m-tiles ----------------
                srow2 = pst("p6", [1, T], name=f"srowm_{l}")
                qrow2 = pst("p7", [1, T], name=f"qrowm_{l}")
                s_list = []
                for c in range(8):
                    psG = [pst("p4", name=f"psG{l}_{c}_0"), pst("p5", name=f"psG{l}_{c}_1")]
                    psU = [pst("p0", name=f"psU{l}_{c}_0"), pst("p1", name=f"psU{l}_{c}_1")]
                    for k in range(KD):
                        wg_t = wmp.tile([128, NTOK], f32r, name=f"wg{l}_{c}_{k}", tag="wg")
                        nc.sync.dma_start(out=wg_t, in_=W1G[l, 128 * k:128 * (k + 1),
                                                          256 * c:256 * (c + 1)])
                        wx_t = wmp.tile([128, NTOK], f32r, name=f"wx{l}_{c}_{k}", tag="wx")
                        nc.sync.dma_start(out=wx_t, in_=W1X[l, 128 * k:128 * (k + 1),
                                                          256 * c:256 * (c + 1)])
                        for j in range(2):
                            nc.tensor.matmul(psG[j], wg_t[:, 128 * j:128 * (j + 1)],
                                             y2[k][:, :], start=(k == 0), stop=(k == KD - 1))
                            nc.tensor.matmul(psU[j], wx_t[:, 128 * j:128 * (j + 1)],
                                             y2[k][:, :], start=(k == 0), stop=(k == KD - 1))
                    for j in range(2):
                        m_idx = 2 * c + j
                        th = sp.tile([128, T], f32, name=f"th{l}_{m_idx}", tag="th")
                        nc.scalar.activation(th[:, :], psG[j][:, :], AF.Tanh, scale=0.5)
                        uc = sp.tile([128, T], f32, name=f"uc{l}_{m_idx}", tag="uc")
                        nc.scalar.copy(uc[:, :], psU[j][:, :])
                        pp = sp.tile([128, T], f32, name=f"pp{l}_{m_idx}", tag="pp")
                        nc.vector.tensor_mul(pp[:, :], psG[j][:, :], uc[:, :])
                        s_ = snp.tile([128, T], f32r, name=f"s{l}_{m_idx}", tag=f"s{m_idx}")
                        nc.vector.scalar_tensor_tensor(s_[:, :], th[:, :], 1.0, pp[:, :],
                                                       op0=OP.add, op1=OP.mult)
                        sq_ = sqp.tile([128, T], f32r, name=f"ssq{l}_{m_idx}",
                                       tag=f"sq{m_idx % 2}")
                        nc.scalar.activation(sq_[:, :], s_[:, :], AF.Square)
                        nc.tensor.matmul(srow2, ones1[:, :], s_[:, :],
                                         start=(m_idx == 0), stop=(m_idx == KH - 1))
                        nc.tensor.matmul(qrow2, ones1[:, :], sq_[:, :],
                                         start=(m_idx == 0), stop=(m_idx == KH - 1))
                        s_list.append(s_)

                if dbg and l == 0:
                    for m_idx in range(KH):
                        nc.sync.dma_start(out=DS[128 * m_idx:128 * (m_idx + 1), :], in_=s_list[m_idx][:, :].bitcast(f32))
                arow, bneg = ln_chain(srow2, qrow2, HIDDEN, f"m{l}")
                Am = pst("p2", name=f"Am_{l}")
                nc.tensor.matmul(Am, onesk[:, :], arow[:, :], start=True, stop=True)
                Bm = pst("p3", name=f"Bm_{l}")
                nc.tensor.matmul(Bm, onesk[:, :], bneg[:, :], start=True, stop=True)
                mN = []
                for m_idx in range(KH):
                    tmp = rtmp.tile([128, T], f32, name=f"mt{l}_{m_idx}", tag="lnt")
                    nc.vector.tensor_add(tmp[:, :], s_list[m_idx][:, :], Bm[:, :])
                    nc.vector.tensor_mul(s_list[m_idx][:, :], tmp[:, :], Am[:, :])
                    mN.append(s_list[m_idx])

                # ---------------- W2 + residual ----------------
                PROJ_TAGS_M = ["p4", "p5", "p0", "p1", "p2", "p3"]
                pss = [pst(PROJ_TAGS_M[m], name=f"psm{l}_{m}") for m in range(KD)]
                for k in range(KH):
                    w2_t = wp.tile([128, DIM], f32r, name=f"w2_{l}_{k}", tag=f"w{k % 2}")
                    nc.sync.dma_start(out=w2_t, in_=W2[l, 128 * k:128 * (k + 1), :])
                    for m in range(KD):
                        nc.tensor.matmul(pss[m], w2_t[:, 128 * m:128 * (m + 1)],
                                         mN[k][:, :], start=(k == 0), stop=(k == KH - 1))
                for m in range(KD):
                    nc.vector.tensor_add(h[m][:, :], h[m][:, :], pss[m][:, :])

            for k in range(KD):
                nc.sync.dma_start(out=out_fm[128 * k:128 * (k + 1), :],
                                  in_=h[k][:, :].bitcast(f32))

    nc.compile()
    return nc


def _prep_host(inputs, layers=DEPTH):
    x = np.asarray(inputs['x'], np.float32)
    pos = np.asarray(inputs['pos_embed'], np.float32)
    rope = np.asarray(inputs['rope_emb'], np.float32)
    g = lambda n: np.asarray(inputs[n], np.float32)

    for n in ('bq', 'bv', 'bo', 'b1g', 'b1x', 'b2', 'ln1_b', 'ln2_b', 'lnm_b'):
        assert np.abs(g(n)).max() == 0.0, f"nonzero bias {n} unsupported"

    ln1w, ln2w, lnmw = g('ln1_w'), g('ln2_w'), g('lnm_w')
    wq = g('wq') * ln1w[:, None, :] * SCALE
    wk = g('wk') * ln1w[:, None, :]
    wv = g('wv') * ln1w[:, None, :]
    wo = g('wo')
    w1g = g('w1g') * ln2w[:, None, :]
    w1x = g('w1x') * ln2w[:, None, :]
    w2 = g('w2') * lnmw[:, None, :]

    tr = lambda w: np.ascontiguousarray(w[:layers].transpose(0, 2, 1))
    WQt, WKt, WVt, WOt = tr(wq), tr(wk), tr(wv), tr(wo)
    W1Gt, W1Xt, W2t = tr(w1g), tr(w1x), tr(w2)

    sinp = np.ascontiguousarray(rope[:, :HD].T)
    cosp = np.ascontiguousarray(rope[:, HD:].T)
    SINt = np.tile(sinp, (2, BPC)).astype(np.float32)
    COSt = np.tile(cosp, (2, BPC)).astype(np.float32)

    p64 = np.zeros((64, 64), np.float32)
    for i2 in range(32):
        p64[2 * i2 + 1, 2 * i2] = -1.0
        p64[2 * i2, 2 * i2 + 1] = 1.0
    PERMt = np.zeros((128, 128), np.float32)
    PERMt[0:64, 0:64] = p64
    PERMt[64:128, 64:128] = p64

    xp = x + pos
    in_maps = []
    for c in range(NCORES):
        xc = xp[BPC * c:BPC * (c + 1)].reshape(T, DIM).T
        in_maps.append({
            "x_fm": np.ascontiguousarray(xc),
            "WQ": WQt, "WK": WKt, "WV": WVt, "WO": WOt,
            "W1G": W1Gt, "W1X": W1Xt, "W2": W2t,
            "ONES": np.ones((1, 128), np.float32),
            "SIN": SINt, "COS": COSt, "PERM": PERMt,
        })
    return in_maps


def kernel(_layers=DEPTH, _trace=False, **inputs):
    _install_ntff_shim()
    from concourse import bass_utils
    if _layers not in _CACHE:
        _CACHE[_layers] = _build(_layers)
    nc = _CACHE[_layers]
    in_maps = _prep_host(inputs, _layers)
    res = bass_utils.run_bass_kernel_spmd(nc, in_maps, core_ids=list(range(NCORES)),
                                          trace=_trace)
    out = np.empty((B, NTOK, DIM), np.float32)
    for c in range(NCORES):
        o = res.results[c]["out_fm"]
        out[BPC * c:BPC * (c + 1)] = o.T.reshape(BPC, NTOK, DIM)
    kernel.last_exec_ns = res.exec_time_ns
    return out


# revision 14
# speedup vs baseline: 1.0994x; 1.0994x over previous
"""Trainium2 Bass kernel for a 12-layer EVA-style ViT encoder (B=16, N=256, D=768).

Sharding: pure data-parallel over batch across 8 NeuronCores (2 images/core).
Per core: feature-major activations [feature, token] (T=512 token columns),
all matmuls float32r (11-bit mantissa at bf16 speed), fp32 residual stream,
LayerNorm affines & biases folded on host (identity/zero for this model).
"""
import sys, types

sys.path.insert(0, '/opt/trn_rl_repo')

import numpy as np

B, NTOK, DIM, HEADS, HD, DEPTH, HIDDEN = 16, 256, 768, 12, 64, 12, 2048
EPS = 1e-5
NCORES = 8
BPC = B // NCORES          # batch items per core
T = BPC * NTOK             # 512 token columns per core
KD = DIM // 128            # 6
KH = HIDDEN // 128         # 16
SCALE = HD ** -0.5

_CACHE = {}


def _install_ntff_shim():
    if "antenv.axon_hooks" in sys.modules:
        return
    m = types.ModuleType("antenv.axon_hooks")
    m._hook = None
    m.set_axon_ntff_profile_hook = lambda h: setattr(m, "_hook", h)
    m.get_axon_ntff_profile_hook = lambda: m._hook
    sys.modules["antenv.axon_hooks"] = m
    try:
        from trn_agent_boot.trn_boot import _ntff_profile_via_ctypes
        m.set_axon_ntff_profile_hook(_ntff_profile_via_ctypes('/opt/axon/libaxon_pjrt.so'))
    except Exception:
        pass


def _build(layers=DEPTH, dbg=False):
    import concourse.bass as bass
    import concourse.mybir as mybir
    import concourse.tile as tile
    from concourse import bacc
    from contextlib import ExitStack

    f32 = mybir.dt.float32
    f32r = mybir.dt.float32r
    i32 = mybir.dt.int32
    AF = mybir.ActivationFunctionType
    OP = mybir.AluOpType

    nc = bacc.Bacc("TRN2", target_bir_lowering=False, debug=False)

    x_fm = nc.dram_tensor("x_fm", [DIM, T], f32r, kind="ExternalInput")
    WQ = nc.dram_tensor("WQ", [layers, DIM, DIM], f32r, kind="ExternalInput")
    WK = nc.dram_tensor("WK", [layers, DIM, DIM], f32r, kind="ExternalInput")
    WV = nc.dram_tensor("WV", [layers, DIM, DIM], f32r, kind="ExternalInput")
    WO = nc.dram_tensor("WO", [layers, DIM, DIM], f32r, kind="ExternalInput")
    W1G = nc.dram_tensor("W1G", [layers, 4, DIM, 512], f32r, kind="ExternalInput")
    W1X = nc.dram_tensor("W1X", [layers, 4, DIM, 512], f32r, kind="ExternalInput")
    W2 = nc.dram_tensor("W2", [layers, HIDDEN, DIM], f32r, kind="ExternalInput")
    SIN = nc.dram_tensor("SIN", [128, T], f32, kind="ExternalInput")
    COS = nc.dram_tensor("COS", [128, T], f32, kind="ExternalInput")
    PERM = nc.dram_tensor("PERM", [128, 128], f32r, kind="ExternalInput")
    ONES = nc.dram_tensor("ONES", [1, 128], f32r, kind="ExternalInput")
    out_fm = nc.dram_tensor("out_fm", [DIM, T], f32, kind="ExternalOutput")
    if dbg:
        DY = nc.dram_tensor("DY", [DIM, T], f32, kind="ExternalOutput")
        DQ = nc.dram_tensor("DQ", [DIM, T], f32, kind="ExternalOutput")
        DK = nc.dram_tensor("DK", [DIM, T], f32, kind="ExternalOutput")
        DV = nc.dram_tensor("DV", [512, DIM], f32, kind="ExternalOutput")
        DE = nc.dram_tensor("DE", [128, 512], f32, kind="ExternalOutput")
        DOT = nc.dram_tensor("DOT", [DIM, T], f32, kind="ExternalOutput")
        DH = nc.dram_tensor("DH", [DIM, T], f32, kind="ExternalOutput")
        DS = nc.dram_tensor("DS", [HIDDEN, T], f32, kind="ExternalOutput")

    with tile.TileContext(nc) as tc:
        with ExitStack() as ctx:
            ctx.enter_context(nc.allow_low_precision(
                reason="float32r tiles hold full fp32 bits; rounding only at PE read"))
            const = ctx.enter_context(tc.tile_pool(name="const", bufs=1))
            hp = ctx.enter_context(tc.tile_pool(name="hp", bufs=1))
            yp = ctx.enter_context(tc.tile_pool(name="yp", bufs=1))
            sqp = ctx.enter_context(tc.tile_pool(name="sqp", bufs=1))
            rowp = ctx.enter_context(tc.tile_pool(name="rowp", bufs=1))
            qrawp = ctx.enter_context(tc.tile_pool(name="qrawp", bufs=2))
            ropep = ctx.enter_context(tc.tile_pool(name="ropep", bufs=1))
            rtmp = ctx.enter_context(tc.tile_pool(name="rtmp", bufs=2))
            vp = ctx.enter_context(tc.tile_pool(name="vp", bufs=1))
            ep = ctx.enter_context(tc.tile_pool(name="ep", bufs=3))
            recp = ctx.enter_context(tc.tile_pool(name="recp", bufs=2))
            op_ = ctx.enter_context(tc.tile_pool(name="op", bufs=1))
            sp = ctx.enter_context(tc.tile_pool(name="sp", bufs=2))
            snp = ctx.enter_context(tc.tile_pool(name="snp", bufs=1))
            wp = ctx.enter_context(tc.tile_pool(name="wp", bufs=2))
            wvp = ctx.enter_context(tc.tile_pool(name="wvp", bufs=1))
            wmp = ctx.enter_context(tc.tile_pool(name="wmp", bufs=3))
            PS = ctx.enter_context(tc.tile_pool(name="PS", bufs=1, space="PSUM"))

            def pst(tag, shape=None, dt_=f32, name=None):
                return PS.tile(shape or [128, T], dt_, name=name or f"ps_{tag}_{nc.next_id()}",
                               tag=tag)

            # constants
            ones1 = const.tile([128, 1], f32r)
            nc.gpsimd.dma_start(out=ones1, in_=bass.AP(ONES.ap().tensor, ONES.ap().offset,
                                                       [[0, 128], [1, 1]]))
            onesk = const.tile([1, 128], f32r)
            nc.sync.dma_start(out=onesk, in_=ONES[:, :])
            magic = const.tile([1, T], i32)
            nc.vector.memset(magic, 0x5f3759df)
            warm8 = const.tile([128, 8], mybir.dt.bfloat16)
            nc.vector.memset(warm8, 1.0)
            sin_sb = const.tile([128, T], f32)
            nc.sync.dma_start(out=sin_sb, in_=SIN[:, :])
            cos_sb = const.tile([128, T], f32)
            nc.sync.dma_start(out=cos_sb, in_=COS[:, :])
            perm_sb = const.tile([128, 128], f32r)
            nc.sync.dma_start(out=perm_sb, in_=PERM[:, :])

            # PE warmup (absorbs entry-barrier waits)
            wps = pst("p7", [8, 8], name="warmps")
            nc.tensor.matmul(wps, warm8[:, :], warm8[:, 0:8], start=True, stop=True)

            h = []
            for k in range(KD):
                t_ = hp.tile([128, T], f32r, name=f"h_{k}", tag=f"h{k}")
                nc.sync.dma_start(out=t_, in_=x_fm[128 * k:128 * (k + 1), :])
                h.append(t_)

            def ln_chain(srow, qrow, D, sfx):
                """DVE scalar chain on [1,T] rows -> (arow=rstd f32r, bneg=-mean f32r)."""
                inv = 1.0 / D
                bneg = rowp.tile([1, T], f32r, name=f"bneg{sfx}", tag="bneg")
                nc.vector.tensor_scalar(bneg[:, :], srow[:, :], -inv, None, op0=OP.mult)
                msq = rowp.tile([1, T], f32, name=f"msq{sfx}", tag="tt")
                nc.vector.tensor_mul(msq[:, :], bneg[:, :], bneg[:, :])
                ve = rowp.tile([1, T], f32, name=f"ve{sfx}", tag="ve")
                nc.vector.scalar_tensor_tensor(ve[:, :], qrow[:, :], inv, msq[:, :],
                                               op0=OP.mult, op1=OP.subtract)
                nc.vector.tensor_scalar(ve[:, :], ve[:, :], EPS, None, op0=OP.add)
                yv = rowp.tile([1, T], i32, name=f"yv{sfx}", tag="yv")
                nc.vector.tensor_scalar(yv[:, :], ve[:, :].bitcast(i32), 1, None,
                                        op0=OP.logical_shift_right)
                nc.vector.scalar_tensor_tensor(yv[:, :], magic[:, :], 1, yv[:, :],
                                               op0=OP.mult, op1=OP.subtract)
                yf = yv[:, :].bitcast(f32)
                tt = rowp.tile([1, T], f32, name=f"tt{sfx}", tag="tt")
                uu = rowp.tile([1, T], f32, name=f"uu{sfx}", tag="uu")
                arow = rowp.tile([1, T], f32r, name=f"arow{sfx}", tag="arow")
                for it in range(2):
                    nc.vector.tensor_mul(tt[:, :], yf, yf)
                    nc.vector.tensor_mul(uu[:, :], tt[:, :], ve[:, :])
                    nc.vector.tensor_scalar(uu[:, :], uu[:, :], -0.5, 1.5,
                                            op0=OP.mult, op1=OP.add)
                    if it == 0:
                        nc.vector.tensor_mul(yv[:, :].bitcast(f32), yf, uu[:, :])
                    else:
                        nc.vector.tensor_mul(arow[:, :], yf, uu[:, :])
                return arow, bneg

            for l in range(layers):
                # ---------------- LN1 ----------------
                sqs = []
                for k in range(KD):
                    s_ = sqp.tile([128, T], f32r, name=f"sq1_{l}_{k}", tag=f"sq{k % 2}")
                    nc.scalar.activation(s_[:, :], h[k][:, :], AF.Square)
                    sqs.append(s_)
                srow = pst("p0", [1, T], name=f"srow1_{l}")
                qrow = pst("p1", [1, T], name=f"qrow1_{l}")
                for k in range(KD):
                    nc.tensor.matmul(srow, ones1[:, :], h[k][:, :],
                                     start=(k == 0), stop=(k == KD - 1))
                for k in range(KD):
                    nc.tensor.matmul(qrow, ones1[:, :], sqs[k][:, :],
                                     start=(k == 0), stop=(k == KD - 1))
                arow, bneg = ln_chain(srow, qrow, DIM, f"a{l}")
                A1 = pst("p2", name=f"A1_{l}")
                nc.tensor.matmul(A1, onesk[:, :], arow[:, :], start=True, stop=True)
                B1 = pst("p3", name=f"B1_{l}")
                nc.tensor.matmul(B1, onesk[:, :], bneg[:, :], start=True, stop=True)
                y1 = []
                for k in range(KD):
                    tmp = rtmp.tile([128, T], f32, name=f"l1t{l}_{k}", tag="lnt")
                    nc.vector.tensor_add(tmp[:, :], h[k][:, :], B1[:, :])
                    y_ = yp.tile([128, T], f32r, name=f"y1_{l}_{k}", tag=f"y{k}")
                    nc.vector.tensor_mul(y_[:, :], tmp[:, :], A1[:, :])
                    y1.append(y_)

                # ---------------- Q,K projections ----------------
                PROJ_TAGS_QK = ["p4", "p5", "p6", "p7", "p0", "p1"]
                qk_sb = {}
                for wname, Wd in (("q", WQ), ("k", WK)):
                    pss = [pst(PROJ_TAGS_QK[m], name=f"ps{wname}{l}_{m}") for m in range(KD)]
                    for k in range(KD):
                        wk_t = wp.tile([128, DIM], f32r, name=f"w{wname}{l}_{k}", tag=f"w{k % 2}")
                        nc.sync.dma_start(out=wk_t, in_=Wd[l, 128 * k:128 * (k + 1), :])
                        for m in range(KD):
                            nc.tensor.matmul(pss[m], wk_t[:, 128 * m:128 * (m + 1)],
                                             y1[k][:, :], start=(k == 0), stop=(k == KD - 1))
                    outs = []
                    for m in range(KD):
                        q_ = qrawp.tile([128, T], f32r, name=f"{wname}sb{l}_{m}", tag=f"qr{m % 2}")
                        nc.scalar.copy(q_[:, :], pss[m][:, :])
                        outs.append(q_)
                    qk_sb[wname] = outs

                # ---------------- V projection (token-major + ones cols) ----------------
                wv_t = []
                for k in range(KD):
                    wv_ = wvp.tile([128, DIM], f32r, name=f"wv{l}_{k}", tag=f"wv{k}")
                    nc.sync.dma_start(out=wv_, in_=WV[l, 128 * k:128 * (k + 1), :])
                    wv_t.append(wv_)
                vtm = []
                for mt in range(4):
                    psvA = pst("p2", [128, 384], name=f"psvA{l}_{mt}")
                    psvB = pst("p3", [128, 384], name=f"psvB{l}_{mt}")
                    for k in range(KD):
                        nc.tensor.matmul(psvA, y1[k][:, 128 * mt:128 * (mt + 1)],
                                         wv_t[k][:, 0:384], start=(k == 0), stop=(k == KD - 1))
                    for k in range(KD):
                        nc.tensor.matmul(psvB, y1[k][:, 128 * mt:128 * (mt + 1)],
                                         wv_t[k][:, 384:768], start=(k == 0), stop=(k == KD - 1))
                    v_ = vp.tile([128, 768], f32r, name=f"vtm{l}_{mt}", tag=f"v{mt}")
                    nc.scalar.copy(v_[:, 0:384], psvA[:, :])
                    nc.scalar.copy(v_[:, 384:768], psvB[:, :])
                    vtm.append(v_)

                # ---------------- RoPE ----------------
                for wname in ("q", "k"):
                    raw = qk_sb[wname]
                    roped = []
                    for m in range(KD):
                        rot = pst("p2" if m % 2 == 0 else "p3", name=f"rot{wname}{l}_{m}")
                        nc.tensor.matmul(rot, perm_sb[:, :], raw[m][:, :],
                                         start=True, stop=True)
                        t1 = rtmp.tile([128, T], f32, name=f"t1{wname}{l}_{m}", tag="t1")
                        nc.vector.tensor_mul(t1[:, :], rot[:, :], sin_sb[:, :])
                        t2 = rtmp.tile([128, T], f32, name=f"t2{wname}{l}_{m}", tag="t2")
                        nc.gpsimd.tensor_mul(t2[:, :], raw[m][:, :], cos_sb[:, :])
                        rp = ropep.tile([128, T], f32r, name=f"{wname}p{l}_{m}",
                                        tag=f"{wname}p{m}")
                        nc.vector.tensor_add(rp[:, :], t1[:, :], t2[:, :])
                        roped.append(rp)
                    qk_sb[wname] = roped
                qs, ks = qk_sb["q"], qk_sb["k"]
                if dbg and l == 0:
                    for k in range(KD):
                        nc.sync.dma_start(out=DY[128 * k:128 * (k + 1), :], in_=y1[k][:, :].bitcast(f32))
                        nc.sync.dma_start(out=DQ[128 * k:128 * (k + 1), :], in_=qs[k][:, :].bitcast(f32))
                        nc.sync.dma_start(out=DK[128 * k:128 * (k + 1), :], in_=ks[k][:, :].bitcast(f32))
                    for mt in range(4):
                        nc.sync.dma_start(out=DV[128 * mt:128 * (mt + 1), :], in_=vtm[mt][:, 0:768].bitcast(f32))

                # ---------------- attention ----------------
                o_sb = [op_.tile([128, T], f32r, name=f"osb{l}_{m}", tag=f"o{m}")
                        for m in range(KD)]
                for i in range(BPC):
                    for hh in range(HEADS):
                        p, off = hh // 2, 64 * (hh % 2)
                        par = (i * HEADS + hh) % 2
                        psS = pst(["p6", "p2"][par], [128, 2 * NTOK], name=f"psS{l}_{i}_{hh}")
                        for kt in range(2):
                            nc.tensor.matmul(
                                psS[:, NTOK * kt:NTOK * (kt + 1)],
                                ks[p][off:off + 64,
                                      256 * i + 128 * kt:256 * i + 128 * (kt + 1)],
                                qs[p][off:off + 64, 256 * i:256 * (i + 1)],
                                start=True, stop=True)
                        eT = ep.tile([128, 2 * NTOK], f32r, name=f"eT{l}_{i}_{hh}", tag="eT")
                        nc.scalar.activation(eT[:, :], psS[:, :], AF.Exp)
                        if dbg and l == 0 and i == 0 and hh == 0:
                            nc.sync.dma_start(out=DE[:, :], in_=eT[:, :].bitcast(f32))
                        # denominator row = colsum of exp (over kt partitions)
                        psD = pst(["p0", "p4"][par], [1, NTOK], name=f"psD{l}_{i}_{hh}")
                        for kt in range(2):
                            nc.tensor.matmul(psD, ones1[:, :],
                                             eT[:, NTOK * kt:NTOK * (kt + 1)],
                                             start=(kt == 0), stop=(kt == 1))
                        recr = recp.tile([1, NTOK], f32r, name=f"recr{l}_{i}_{hh}", tag="recr")
                        nc.vector.reciprocal(recr[:, :], psD[:, :])
                        psR = pst(["p1", "p5"][par], [64, NTOK], name=f"psR{l}_{i}_{hh}")
                        nc.tensor.matmul(psR, onesk[:, 0:64], recr[:, :],
                                         start=True, stop=True)
                        rec = recp.tile([64, NTOK], f32, name=f"rec{l}_{i}_{hh}", tag="rec")
                        nc.scalar.copy(rec[:, :], psR[:, :])
                        psO = pst(["p7", "p3"][par], [64, NTOK], name=f"psO{l}_{i}_{hh}")
                        for kt in range(2):
                            vt = vtm[2 * i + kt]
                            nc.tensor.matmul(psO, vt[:, 64 * hh:64 * hh + 64],
                                             eT[:, NTOK * kt:NTOK * (kt + 1)],
                                             start=(kt == 0), stop=(kt == 1))
                        nc.vector.tensor_mul(
                            o_sb[p][off:off + 64, 256 * i:256 * (i + 1)],
                            psO[:, :], rec[:, :])

                # ---------------- O projection + residual ----------------
                PROJ_TAGS_O = ["p0", "p1", "p2", "p3", "p4", "p5"]
                pss = [pst(PROJ_TAGS_O[m], name=f"psh{l}_{m}") for m in range(KD)]
                for k in range(KD):
                    wo_t = wp.tile([128, DIM], f32r, name=f"wo{l}_{k}", tag=f"w{k % 2}")
                    nc.sync.dma_start(out=wo_t, in_=WO[l, 128 * k:128 * (k + 1), :])
                    for m in range(KD):
                        nc.tensor.matmul(pss[m], wo_t[:, 128 * m:128 * (m + 1)],
                                         o_sb[k][:, :], start=(k == 0), stop=(k == KD - 1))
                for m in range(KD):
                    nc.vector.tensor_add(h[m][:, :], h[m][:, :], pss[m][:, :])

                if dbg and l == 0:
                    for k in range(KD):
                        nc.sync.dma_start(out=DOT[128 * k:128 * (k + 1), :], in_=o_sb[k][:, :].bitcast(f32))
                        nc.sync.dma_start(out=DH[128 * k:128 * (k + 1), :], in_=h[k][:, :].bitcast(f32))
                # ---------------- LN2 ----------------
                sqs = []
                for k in range(KD):
                    s_ = sqp.tile([128, T], f32r, name=f"sq2_{l}_{k}", tag=f"sq{k % 2}")
                    nc.scalar.activation(s_[:, :], h[k][:, :], AF.Square)
                    sqs.append(s_)
                srow = pst("p6", [1, T], name=f"srow2_{l}")
                qrow = pst("p7", [1, T], name=f"qrow2_{l}")
                for k in range(KD):
                    nc.tensor.matmul(srow, ones1[:, :], h[k][:, :],
                                     start=(k == 0), stop=(k == KD - 1))
                for k in range(KD):
                    nc.tensor.matmul(qrow, ones1[:, :], sqs[k][:, :],
                                     start=(k == 0), stop=(k == KD - 1))
                arow, bneg = ln_chain(srow, qrow, DIM, f"b{l}")
                A2 = pst("p2", name=f"A2_{l}")
                nc.tensor.matmul(A2, onesk[:, :], arow[:, :], start=True, stop=True)
                B2 = pst("p3", name=f"B2_{l}")
                nc.tensor.matmul(B2, onesk[:, :], bneg[:, :], start=True, stop=True)
                y2 = []
                for k in range(KD):
                    tmp = rtmp.tile([128, T], f32, name=f"l2t{l}_{k}", tag="lnt")
                    nc.vector.tensor_add(tmp[:, :], h[k][:, :], B2[:, :])
                    y_ = yp.tile([128, T], f32r, name=f"y2_{l}_{k}", tag=f"y{k}")
                    nc.vector.tensor_mul(y_[:, :], tmp[:, :], A2[:, :])
                    y2.append(y_)

                # ---------------- MLP G/U in chunks of 2 m-tiles ----------------
                srow2 = pst("p6", [1, T], name=f"srowm_{l}")
                qrow2 = pst("p7", [1, T], name=f"qrowm_{l}")
                s_list = []
                for c4 in range(4):
                    for cc in range(2):
                        c = 2 * c4 + cc
                        psG = [pst("p4", name=f"psG{l}_{c}_0"), pst("p5", name=f"psG{l}_{c}_1")]
                        psU = [pst("p0", name=f"psU{l}_{c}_0"), pst("p1", name=f"psU{l}_{c}_1")]
                        for k in range(KD):
                            wg_t = wmp.tile([128, NTOK], f32r, name=f"wg{l}_{c}_{k}", tag="wg")
                            nc.sync.dma_start(out=wg_t, in_=W1G[l, c4, 128 * k:128 * (k + 1),
                                                              256 * cc:256 * (cc + 1)])
                            wx_t = wmp.tile([128, NTOK], f32r, name=f"wx{l}_{c}_{k}", tag="wx")
                            nc.sync.dma_start(out=wx_t, in_=W1X[l, c4, 128 * k:128 * (k + 1),
                                                              256 * cc:256 * (cc + 1)])
                            for j in range(2):
                                nc.tensor.matmul(psG[j], wg_t[:, 128 * j:128 * (j + 1)],
                                                 y2[k][:, :], start=(k == 0), stop=(k == KD - 1))
                                nc.tensor.matmul(psU[j], wx_t[:, 128 * j:128 * (j + 1)],
                                                 y2[k][:, :], start=(k == 0), stop=(k == KD - 1))
                        for j in range(2):
                            m_idx = 2 * c + j
                            th = sp.tile([128, T], f32, name=f"th{l}_{m_idx}", tag="th")
                            nc.scalar.activation(th[:, :], psG[j][:, :], AF.Tanh, scale=0.5)
                            uc = sp.tile([128, T], f32, name=f"uc{l}_{m_idx}", tag="uc")
                            nc.scalar.copy(uc[:, :], psU[j][:, :])
                            pp = sp.tile([128, T], f32, name=f"pp{l}_{m_idx}", tag="pp")
                            nc.vector.tensor_mul(pp[:, :], psG[j][:, :], uc[:, :])
                            s_ = snp.tile([128, T], f32r, name=f"s{l}_{m_idx}", tag=f"s{m_idx}")
                            nc.vector.scalar_tensor_tensor(s_[:, :], th[:, :], 1.0, pp[:, :],
                                                           op0=OP.add, op1=OP.mult)
                            sq_ = sqp.tile([128, T], f32r, name=f"ssq{l}_{m_idx}",
                                           tag=f"sq{m_idx % 2}")
                            nc.scalar.activation(sq_[:, :], s_[:, :], AF.Square)
                            nc.tensor.matmul(srow2, ones1[:, :], s_[:, :],
                                             start=(m_idx == 0), stop=(m_idx == KH - 1))
                            nc.tensor.matmul(qrow2, ones1[:, :], sq_[:, :],
                                             start=(m_idx == 0), stop=(m_idx == KH - 1))
                            s_list.append(s_)

                if dbg and l == 0:
                    for m_idx in range(KH):
                        nc.sync.dma_start(out=DS[128 * m_idx:128 * (m_idx + 1), :], in_=s_list[m_idx][:, :].bitcast(f32))
                arow, bneg = ln_chain(srow2, qrow2, HIDDEN, f"m{l}")
                Am = pst("p2", name=f"Am_{l}")
                nc.tensor.matmul(Am, onesk[:, :], arow[:, :], start=True, stop=True)
                Bm = pst("p3", name=f"Bm_{l}")
                nc.tensor.matmul(Bm, onesk[:, :], bneg[:, :], start=True, stop=True)
                mN = []
                for m_idx in range(KH):
                    tmp = rtmp.tile([128, T], f32, name=f"mt{l}_{m_idx}", tag="lnt")
                    nc.vector.tensor_add(tmp[:, :], s_list[m_idx][:, :], Bm[:, :])
                    nc.vector.tensor_mul(s_list[m_idx][:, :], tmp[:, :], Am[:, :])
                    mN.append(s_list[m_idx])

                # ---------------- W2 + residual ----------------
                PROJ_TAGS_M = ["p4", "p5", "p0", "p1", "p2", "p3"]
                pss = [pst(PROJ_TAGS_M[m], name=f"psm{l}_{m}") for m in range(KD)]
                for k in range(KH):
                    w2_t = wp.tile([128, DIM], f32r, name=f"w2_{l}_{k}", tag=f"w{k % 2}")
                    nc.sync.dma_start(out=w2_t, in_=W2[l, 128 * k:128 * (k + 1), :])
                    for m in range(KD):
                        nc.tensor.matmul(pss[m], w2_t[:, 128 * m:128 * (m + 1)],
                                         mN[k][:, :], start=(k == 0), stop=(k == KH - 1))
                for m in range(KD):
                    nc.vector.tensor_add(h[m][:, :], h[m][:, :], pss[m][:, :])

            for k in range(KD):
                nc.sync.dma_start(out=out_fm[128 * k:128 * (k + 1), :],
                                  in_=h[k][:, :].bitcast(f32))

    nc.compile()
    return nc


def _prep_host(inputs, layers=DEPTH):
    x = np.asarray(inputs['x'], np.float32)
    pos = np.asarray(inputs['pos_embed'], np.float32)
    rope = np.asarray(inputs['rope_emb'], np.float32)
    g = lambda n: np.asarray(inputs[n], np.float32)

    for n in ('bq', 'bv', 'bo', 'b1g', 'b1x', 'b2', 'ln1_b', 'ln2_b', 'lnm_b'):
        assert np.abs(g(n)).max() == 0.0, f"nonzero bias {n} unsupported"

    ln1w, ln2w, lnmw = g('ln1_w'), g('ln2_w'), g('lnm_w')
    wq = g('wq') * ln1w[:, None, :] * SCALE
    wk = g('wk') * ln1w[:, None, :]
    wv = g('wv') * ln1w[:, None, :]
    wo = g('wo')
    w1g = g('w1g') * ln2w[:, None, :]
    w1x = g('w1x') * ln2w[:, None, :]
    w2 = g('w2') * lnmw[:, None, :]

    tr = lambda w: np.ascontiguousarray(w[:layers].transpose(0, 2, 1))
    WQt, WKt, WVt, WOt = tr(wq), tr(wk), tr(wv), tr(wo)
    W2t = tr(w2)
    # chunk-major [L, 4, DIM, 512] so DMA rows are 2KB contiguous
    W1Gt = np.ascontiguousarray(tr(w1g).reshape(layers, DIM, 4, 512).transpose(0, 2, 1, 3))
    W1Xt = np.ascontiguousarray(tr(w1x).reshape(layers, DIM, 4, 512).transpose(0, 2, 1, 3))

    sinp = np.ascontiguousarray(rope[:, :HD].T)
    cosp = np.ascontiguousarray(rope[:, HD:].T)
    SINt = np.tile(sinp, (2, BPC)).astype(np.float32)
    COSt = np.tile(cosp, (2, BPC)).astype(np.float32)

    p64 = np.zeros((64, 64), np.float32)
    for i2 in range(32):
        p64[2 * i2 + 1, 2 * i2] = -1.0
        p64[2 * i2, 2 * i2 + 1] = 1.0
    PERMt = np.zeros((128, 128), np.float32)
    PERMt[0:64, 0:64] = p64
    PERMt[64:128, 64:128] = p64

    xp = x + pos
    in_maps = []
    for c in range(NCORES):
        xc = xp[BPC * c:BPC * (c + 1)].reshape(T, DIM).T
        in_maps.append({
            "x_fm": np.ascontiguousarray(xc),
            "WQ": WQt, "WK": WKt, "WV": WVt, "WO": WOt,
            "W1G": W1Gt, "W1X": W1Xt, "W2": W2t,
            "ONES": np.ones((1, 128), np.float32),
            "SIN": SINt, "COS": COSt, "PERM": PERMt,
        })
    return in_maps


def kernel(_layers=DEPTH, _trace=False, **inputs):
    _install_ntff_shim()
    from concourse import bass_utils
    if _layers not in _CACHE:
        _CACHE[_layers] = _build(_layers)
    nc = _CACHE[_layers]
    in_maps = _prep_host(inputs, _layers)
    res = bass_utils.run_bass_kernel_spmd(nc, in_maps, core_ids=list(range(NCORES)),
                                          trace=_trace)
    out = np.empty((B, NTOK, DIM), np.float32)
    for c in range(NCORES):
        o = res.results[c]["out_fm"]
        out[BPC * c:BPC * (c + 1)] = o.T.reshape(BPC, NTOK, DIM)
    kernel.last_exec_ns = res.exec_time_ns
    return out
